# revision 1
# baseline (speedup 1.0000x reference)
"""DA-RNN (dual-stage attention RNN) forward pass on 8 TRN2 NeuronCores.

Data-parallel: batch 2048 sharded 256 per core, weights replicated.

Algebraic structure exploited (validated against the reference in numpy):
  * Both attention blocks add their state-dependent term as a per-sample
    constant across the softmax axis, so softmax cancels it.  The encoder
    input attention (a1, a2) and the decoder temporal attention (beta) are
    therefore input-only precomputes, and the decoder context vector is
    constant across decoder steps.
  * context only enters through dot products (fc_W, fc_final_W), so it is
    never materialized: three matvec columns [v, fcW1, w_c] against
    X_encoded give score/q/r per (sample, t), and softmax-weighted sums of
    q, r give the decoder LSTM input offset and the output contribution.
  * The decoder LSTM input is scalar per sample, so its input matmul is a
    K=2 augmented matmul (value row + ones row carrying the bias).

On-chip layout: feature-major [dim on partitions (128-chunks), batch on
free].  LSTM states stay in that layout so no transposes in the recurrences.
Compute dtype fp16 for matmuls (PSUM accumulates fp32; cell c state fp32).
"""

import sys
import os

sys.path.insert(0, "/opt/trn_rl_repo")
os.environ.setdefault("MYCRO_LOCAL_CACHE", "1")

from contextlib import ExitStack

import numpy as np
import ml_dtypes

import concourse.bass as bass
import concourse.mybir as mybir
import concourse.tile as tile
from concourse import bacc
from concourse.bass_utils import run_bass_kernel_spmd
from concourse.masks import make_identity

F32 = mybir.dt.float32
BF16 = mybir.dt.bfloat16
AF = mybir.ActivationFunctionType
ALU = mybir.AluOpType

NCORES = 8
P = 128
BS = 256          # batch per core
NJ = 2            # 128-partition batch chunks
T = 9             # recurrence steps (T-1 in the reference)
H = 512
IN2 = 15
ME = 16           # encoder gate chunks (4H/128)
KE = 4            # encoder hidden chunks (H/128)
MD = 32           # decoder gate chunks (8H/128)
KD = 8            # decoder hidden chunks (2H/128)

CDT = mybir.dt.float16   # matmul compute dtype (streams 1 cycle/row, fp16 mantissa)
GDT = mybir.dt.float16   # gate tiles (DVE-only)
N_WARM_PREFIX = 48       # dummy PE warm-up matmuls during the DMA/precompute prefix
N_WARM_MID = 36          # dummy PE warm-keeper matmuls across the attention gap


def _np(a):
    return np.asarray(a, dtype=np.float32)


def _bf(a):
    return np.ascontiguousarray(np.asarray(a, dtype=np.float32).astype(np.float16))


def _pack_weights(inp):
    """Host-side weight folding (weight-only transforms; no input math)."""
    Wih1, Whh1 = _np(inp["enc_lstm_Wih"]), _np(inp["enc_lstm_Whh"])
    b1 = _np(inp["enc_lstm_bih"]) + _np(inp["enc_lstm_bhh"])
    Wih2, Whh2 = _np(inp["enc_lstm1_Wih"]), _np(inp["enc_lstm1_Whh"])
    b2 = _np(inp["enc_lstm1_bih"]) + _np(inp["enc_lstm1_bhh"])
    Wd_ih, Wd_hh = _np(inp["dec_lstm_Wih"]), _np(inp["dec_lstm_Whh"])
    bd = _np(inp["dec_lstm_bih"]) + _np(inp["dec_lstm_bhh"])
    attn1_W = _np(inp["dec_attn1_W"])
    attn2_w = _np(inp["dec_attn2_W"])[0]
    fc_W = _np(inp["fc_W"])[0]
    fcf_W = _np(inp["fc_final_W"])[0]

    W1x = attn1_W[:, 4 * H:]                        # (512, 1024)
    v = W1x.T @ attn2_w                             # (1024,)
    fcW1 = fc_W[:2 * H]
    w_c = fcf_W[2 * H:]
    w_d = fcf_W[:2 * H]

    weights = {
        "wia1": _bf(np.concatenate([Wih1.T, b1[None, :]], axis=0)),     # (17, 2048)
        "wia2": _bf(np.concatenate([Wih2.T, b2[None, :]], axis=0)),     # (16, 2048)
        "whh1": _bf(Whh1.T),                                            # (512, 2048)
        "whh2": _bf(Whh2.T),                                            # (512, 2048)
        "whhd": _bf(Wd_hh.T),                                           # (1024, 4096)
        "wid": _bf(np.stack([Wd_ih[:, 0],
                             bd + Wd_ih[:, 0] * float(_np(inp["fc_b"])[0])],
                            axis=0)),                                       # (2, 4096)
        "v3": _bf(np.stack([v, fcW1, w_c], axis=1)),                    # (1024, 3)
        "wd": _bf(w_d[:, None]),                                        # (1024, 1)
    }
    scalars = {
        "Wf": [float(x) for x in _np(inp["enc_attn_W"])[0, 2 * H:]],    # 9 floats
        "w_y": float(fc_W[2 * H]),
        "fc_b": float(_np(inp["fc_b"])[0]),
        "fcf_b": float(_np(inp["fc_final_b"])[0]),
    }
    return weights, scalars


def _build(scal, upto="full"):
    nc = bacc.Bacc()

    xd = nc.declare_dram_parameter("x", [BS, T, IN2], F32, isOutput=False)
    yd = nc.declare_dram_parameter("y", [BS, T], F32, isOutput=False)
    wia1d = nc.declare_dram_parameter("wia1", [17, 4 * H], CDT, isOutput=False)
    wia2d = nc.declare_dram_parameter("wia2", [16, 4 * H], CDT, isOutput=False)
    whh1d = nc.declare_dram_parameter("whh1", [H, 4 * H], CDT, isOutput=False)
    whh2d = nc.declare_dram_parameter("whh2", [H, 4 * H], CDT, isOutput=False)
    whhdd = nc.declare_dram_parameter("whhd", [2 * H, 8 * H], CDT, isOutput=False)
    widd = nc.declare_dram_parameter("wid", [2, 8 * H], CDT, isOutput=False)
    v3d = nc.declare_dram_parameter("v3", [2 * H, 3], CDT, isOutput=False)
    wdd = nc.declare_dram_parameter("wd", [2 * H, 1], CDT, isOutput=False)
    outd = nc.declare_dram_parameter("out", [BS, 1], F32, isOutput=True)

    Wf = scal["Wf"]

    with ExitStack() as ctx:
        tc = ctx.enter_context(tile.TileContext(nc))
        # persistent pools
        pw = ctx.enter_context(tc.tile_pool(name="pw", bufs=1))
        psm = ctx.enter_context(tc.tile_pool(name="psm", bufs=4))     # small f32 scratch
        pu = ctx.enter_context(tc.tile_pool(name="pu", bufs=4))       # cell temp
        pya = ctx.enter_context(tc.tile_pool(name="pya", bufs=T))
        psum_g = ctx.enter_context(tc.tile_pool(name="psum_g", bufs=5, space="PSUM"))
        psum_t = ctx.enter_context(tc.tile_pool(name="psum_t", bufs=1, space="PSUM"))
        psum_q = ctx.enter_context(tc.tile_pool(name="psum_q", bufs=1, space="PSUM"))
        psum_f = ctx.enter_context(tc.tile_pool(name="psum_f", bufs=1, space="PSUM"))

        # ---------------- persistent weights / constants ----------------
        yb = pw.tile([P, NJ, T], F32)
        nc.sync.dma_start(out=yb, in_=yd.rearrange("(j p) t -> p j t", p=P))
        xb0 = pw.tile([P, NJ, T, IN2], F32)
        xd_r = xd.rearrange("(j p) t f -> p j t f", p=P)
        for j in range(NJ):
            nc.sync.dma_start(out=xb0[:, j, :, :], in_=xd_r[:, j, :, :])
        ident = pw.tile([P, P], F32)
        make_identity(nc, ident)
        # PE warm-up: back-to-back dummy matmuls during the DMA/precompute
        # prefix so the HAM clock gate reaches 8/8 before the encoder starts.
        wps = psum_f.tile([P, P], F32, tag="psf")

        def _warm(n, nn=64):
            for _ in range(n):
                nc.tensor.matmul(wps[:, 0:nn], ident, ident[:, 0:nn],
                                 start=True, stop=True)

        for _ in range(N_WARM_PREFIX):
            nc.tensor.matmul(wps, ident, ident, start=True, stop=True)
        whhd = pw.tile([P, KD, 8 * H], CDT)
        nc.sync.dma_start(out=whhd, in_=whhdd.rearrange("(k p) m -> p k m", p=P))
        widt = pw.tile([2, 8 * H], CDT)
        nc.sync.dma_start(out=widt, in_=widd[:, :])
        v3t = pw.tile([P, KD, 3], CDT)
        nc.sync.dma_start(out=v3t, in_=v3d.rearrange("(k p) c -> p k c", p=P))
        wdt = pw.tile([P, KD, 1], CDT)
        nc.sync.dma_start(out=wdt, in_=wdd.rearrange("(k p) c -> p k c", p=P))
        sqr = pw.tile([P, NJ, T, 3], F32)
        A_t = pw.tile([P, NJ], F32)
        ctxw = pw.tile([P, NJ], F32)
        ytld2 = pw.tile([P, NJ, 2 * T], F32)
        ytldT = pw.tile([2 * T, BS], CDT)
        cd = pw.tile([P, KD, BS], F32)
        osb = pw.tile([P, NJ, 1], F32)

        with tc.tile_pool(name="penc", bufs=1) as penc:
            # encoder-lifetime weights and states
            wia1 = penc.tile([17, 4 * H], CDT)
            nc.sync.dma_start(out=wia1, in_=wia1d[:, :])
            wia2 = penc.tile([16, 4 * H], CDT)
            nc.sync.dma_start(out=wia2, in_=wia2d[:, :])
            whh1 = penc.tile([P, KE, 4 * H], CDT)
            nc.sync.dma_start(out=whh1, in_=whh1d.rearrange("(k p) m -> p k m", p=P))
            whh2 = penc.tile([P, KE, 4 * H], CDT)
            nc.sync.dma_start(out=whh2, in_=whh2d.rearrange("(k p) m -> p k m", p=P))
            c1 = penc.tile([P, KE, BS], F32)
            c2 = penc.tile([P, KE, BS], F32)
            # per-step transposed inputs [feature rows | ones row], fp16
            xtA1 = [penc.tile([17, BS], CDT, name=f"xa1_{t}", tag=f"xa1_{t}")
                    for t in range(T)]
            xtA2 = [penc.tile([16, BS], CDT, name=f"xa2_{t}", tag=f"xa2_{t}")
                    for t in range(T)]

            with tc.tile_pool(name="ptmp", bufs=1) as ptmp:
                # ---------------- encoder attention precompute ----------------
                # (xt1/xt2 stay live through the encoder: each step's input
                # transposes are traced inside the loop so they land in the
                # dense matmul stream instead of one sparse PE chain.)
                xyb = ptmp.tile([P, NJ, T, 16], F32)
                nc.vector.tensor_copy(out=xyb[:, :, :, 0:IN2], in_=xb0)
                nc.vector.tensor_copy(out=xyb[:, :, :, IN2], in_=yb[:, :, :])
                mmb = ptmp.tile([P, NJ, T, IN2], F32)
                nc.vector.tensor_mul(
                    out=mmb,
                    in0=xyb[:, :, :, 0:IN2],
                    in1=yb.unsqueeze(3).to_broadcast([P, NJ, T, IN2]),
                )
                base1 = ptmp.tile([P, NJ, 16], F32)
                base2 = ptmp.tile([P, NJ, IN2], F32)
                nc.vector.tensor_scalar_mul(out=base1, in0=xyb[:, :, 0, :], scalar1=Wf[0])
                nc.vector.tensor_scalar_mul(out=base2, in0=mmb[:, :, 0, :], scalar1=Wf[0])
                for t in range(1, T):
                    nc.vector.scalar_tensor_tensor(
                        out=base1, in0=xyb[:, :, t, :], scalar=Wf[t], in1=base1,
                        op0=ALU.mult, op1=ALU.add)
                    nc.vector.scalar_tensor_tensor(
                        out=base2, in0=mmb[:, :, t, :], scalar=Wf[t], in1=base2,
                        op0=ALU.mult, op1=ALU.add)

                a1 = ptmp.tile([P, NJ, 16], F32)
                a2 = ptmp.tile([P, NJ, IN2], F32)
                for (base, a) in ((base1, a1), (base2, a2)):
                    for j in range(NJ):
                        ssum = psm.tile([P, 1], F32, tag="ssum")
                        nc.scalar.activation(out=a[:, j, :], in_=base[:, j, :],
                                             func=AF.Exp, accum_out=ssum)
                        inv = psm.tile([P, 1], F32, tag="inv")
                        nc.vector.reciprocal(out=inv, in_=ssum)
                        nc.vector.tensor_scalar_mul(out=a[:, j, :], in0=a[:, j, :],
                                                    scalar1=inv)

                # last column = 1.0 so the transpose yields the ones row that
                # carries the bias through the K-augmented matmul
                xt1 = penc.tile([P, NJ, T, 17], F32)
                nc.vector.memset(xt1[:, :, :, 16:17], 1.0)
                nc.vector.tensor_mul(
                    out=xt1[:, :, :, 0:16], in0=xyb,
                    in1=a1.unsqueeze(2).to_broadcast([P, NJ, T, 16]))
                xt2 = penc.tile([P, NJ, T, 16], F32)
                nc.vector.memset(xt2[:, :, :, IN2:16], 1.0)
                nc.vector.tensor_mul(
                    out=xt2[:, :, :, 0:IN2], in0=mmb,
                    in1=a2.unsqueeze(2).to_broadcast([P, NJ, T, IN2]))

                for t in range(T):
                    for j in range(NJ):
                        tp1 = psum_t.tile([17, P], F32, tag="pst")
                        nc.tensor.transpose(tp1, xt1[:, j, t, :], ident)
                        nc.scalar.copy(out=xtA1[t][:, j * P:(j + 1) * P], in_=tp1)
                        tp2 = psum_t.tile([16, P], F32, tag="pst")
                        nc.tensor.transpose(tp2, xt2[:, j, t, :], ident)
                        nc.vector.tensor_copy(out=xtA2[t][:, j * P:(j + 1) * P],
                                              in_=tp2)
                        if t < 1:
                            _warm(2)

                if upto == "pre":
                    nc.vector.tensor_copy(out=osb, in_=xt1[:, :, 0, 0:1])

                for t in range(T):
                    for j in range(NJ):
                        tp1 = psum_t.tile([17, P], F32, tag="pst")
                        nc.tensor.transpose(tp1, xt1[:, j, t, :], ident)
                        nc.scalar.copy(out=xtA1[t][:, j * P:(j + 1) * P], in_=tp1)
                        tp2 = psum_t.tile([16, P], F32, tag="pst")
                        nc.tensor.transpose(tp2, xt2[:, j, t, :], ident)
                        nc.vector.tensor_copy(out=xtA2[t][:, j * P:(j + 1) * P],
                                              in_=tp2)

            # ---------------- encoder recurrence + score matvecs ----------------
            # State is produced in hidden-chunk PAIRS so that next-step
            # matmuls (which consume one 128-chunk of state per K step) start
            # as soon as their chunk is ready — keeps TensorE busy across the
            # step boundary so the HAM clock gate stays warm.
            with tc.tile_pool(name="px", bufs=3) as px, \
                 tc.tile_pool(name="pg", bufs=4) as pg:
                psq = psum_q.tile([P, NJ, T, 3], F32)
                prev = None  # list of 4 (P, 2, BS) pair tiles: [br1 kp0, br1 kp1, br2 kp0, br2 kp1]
                for t in range(T if upto != "pre" else 0):
                    xe = [px.tile([P, 2, BS], CDT, name=f"xe{i}", tag=f"xe{i}") for i in range(4)]
                    for br, (wia, whh, cbr, koff) in enumerate((
                            (wia1, whh1, c1, 0),
                            (wia2, whh2, c2, KE))):
                        xtA = xtA1[t] if br == 0 else xtA2[t]
                        for kp in range(KE // 2):
                            # gate-pair tile: [gate(i,f,g,o), chunk-in-pair, batch]
                            gt = pg.tile([P, 4, 2, BS], GDT, tag="ge")
                            for gate in range(4):
                                ps = psum_g.tile([P, 2, BS], F32, tag="psg")
                                for half in range(2):
                                    m = gate * KE + 2 * kp + half
                                    nc.tensor.matmul(ps[:, half, :],
                                                     wia[:, m * P:(m + 1) * P],
                                                     xtA[:, :],
                                                     start=True, stop=(t == 0))
                                    if t > 0:
                                        for k in range(KE):
                                            nc.tensor.matmul(
                                                ps[:, half, :],
                                                whh[:, k, m * P:(m + 1) * P],
                                                prev[2 * br + k // 2][:, k % 2, :],
                                                start=False,
                                                stop=(k == KE - 1))
                                fn = AF.Tanh if gate == 2 else AF.Sigmoid
                                nc.scalar.activation(out=gt[:, gate, :, :],
                                                     in_=ps, func=fn)
                            # cell for this pair of hidden chunks
                            cs = cbr[:, 2 * kp:2 * kp + 2, :]
                            if t == 0:
                                nc.vector.tensor_mul(out=cs, in0=gt[:, 0, :, :],
                                                     in1=gt[:, 2, :, :])
                            else:
                                u = pu.tile([P, 2, BS], F32, tag="u")
                                nc.vector.tensor_mul(out=u, in0=gt[:, 0, :, :],
                                                     in1=gt[:, 2, :, :])
                                nc.vector.tensor_mul(out=cs, in0=gt[:, 1, :, :],
                                                     in1=cs)
                                nc.vector.tensor_add(out=cs, in0=cs, in1=u)
                            nc.scalar.activation(out=gt[:, 2, :, :], in_=cs,
                                                 func=AF.Tanh)
                            nc.vector.tensor_mul(out=xe[2 * br + kp],
                                                 in0=gt[:, 3, :, :],
                                                 in1=gt[:, 2, :, :])
                    # score/q/r matvecs against the 3 packed columns
                    for j in range(NJ):
                        for k in range(KD):
                            nc.tensor.matmul(psq[:, j, t, :],
                                             xe[k // 2][:, k % 2, j * P:(j + 1) * P],
                                             v3t[:, k, :],
                                             start=(k == 0), stop=(k == KD - 1))
                    prev = xe

                # ---------------- decoder attention / ytld ----------------
                if upto != "pre":
                    nc.scalar.copy(out=sqr, in_=psq)
                if upto == "enc":
                    nc.vector.tensor_copy(out=osb, in_=sqr[:, :, 0, 0:1])
                for j in range(NJ if upto in ("beta", "ytld", "dec", "dec1", "dec2", "full") else 0):
                    beta = psm.tile([P, T], F32, tag="beta")
                    ssum = psm.tile([P, 1], F32, tag="ssum")
                    nc.scalar.activation(out=beta, in_=sqr[:, j, :, 0], func=AF.Exp,
                                         accum_out=ssum)
                    inv = psm.tile([P, 1], F32, tag="inv")
                    nc.vector.reciprocal(out=inv, in_=ssum)
                    tmp9 = psm.tile([P, T], F32, tag="tmp9")
                    eq = psm.tile([P, 1], F32, tag="eq")
                    nc.vector.tensor_mul(out=tmp9, in0=beta, in1=sqr[:, j, :, 1])
                    nc.vector.reduce_sum(out=eq, in_=tmp9,
                                         axis=mybir.AxisListType.X)
                    nc.vector.tensor_scalar_mul(out=A_t[:, j:j + 1], in0=eq,
                                                scalar1=inv)
                    tmp9b = psm.tile([P, T], F32, tag="tmp9")
                    er = psm.tile([P, 1], F32, tag="eq")
                    nc.vector.tensor_mul(out=tmp9b, in0=beta, in1=sqr[:, j, :, 2])
                    nc.vector.reduce_sum(out=er, in_=tmp9b,
                                         axis=mybir.AxisListType.X)
                    nc.vector.tensor_scalar_mul(out=ctxw[:, j:j + 1], in0=er,
                                                scalar1=inv)

                # ytld interleaved with ones rows, transposed; per-t (2, BS) rhs
                if upto in ("ytld", "dec", "dec1", "dec2", "full"):
                    nc.vector.memset(ytld2, 1.0)
                    for j in range(NJ):
                        nc.vector.tensor_scalar_mul(out=ytld2[:, j, 0:2 * T:2],
                                                    in0=yb[:, j, :],
                                                    scalar1=scal["w_y"])
                        nc.vector.tensor_scalar_add(out=ytld2[:, j, 0:2 * T:2],
                                                    in0=ytld2[:, j, 0:2 * T:2],
                                                    scalar1=A_t[:, j:j + 1])
                    for j in range(NJ):
                        tpy = psum_t.tile([2 * T, P], F32, tag="pst")
                        nc.tensor.transpose(tpy, ytld2[:, j, :], ident)
                        nc.scalar.copy(out=ytldT[:, j * P:(j + 1) * P], in_=tpy)

        # keep the PE warm across the attention/ytld gap (no real PE work
        # there): dummy matmuls gated on the score evacuation.
        if upto in ("dec", "full"):
            wps2 = psum_f.tile([P, P], F32, tag="psf")
            # first link gated on the evacuated scores; the rest chain in
            # PE program order
            nc.tensor.matmul(wps2[:, 0:2 * T * 3], ident,
                             sqr.rearrange("p a b c -> p (a b c)"),
                             start=True, stop=True)
            for _ in range(N_WARM_MID):
                nc.tensor.matmul(wps2, ident, ident, start=True, stop=True)

        ytA = []
        for t in range(T if upto in ("ytld", "dec", "dec1", "dec2", "full") else 0):
            yt = pya.tile([2, BS], CDT, tag="ytA")
            nc.sync.dma_start(out=yt, in_=ytldT[2 * t:2 * t + 2, :])
            ytA.append(yt)
        if upto in ("beta", "ytld"):
            nc.vector.tensor_copy(out=osb, in_=A_t.unsqueeze(2))

        # ---------------- decoder recurrence ----------------
        # Same pair-sliced structure as the encoder (see comment there).
        # State pair tiles are freshly allocated each step (pool-rotated):
        # later pair-groups of step t still read the OLD state, so in-place
        # updates would be a WAR hazard.
        ndec = T if upto in ("dec", "full") else (1 if upto == "dec1" else (2 if upto == "dec2" else 0))
        dTp = None
        pdt = ctx.enter_context(tc.tile_pool(name="pdt", bufs=3))
        with tc.tile_pool(name="pgd", bufs=6) as pgd:
            for t in range(ndec):
                dprev = dTp
                dTp = [pdt.tile([P, 2, BS], CDT, name=f"dT{i}", tag=f"dT{i}")
                       for i in range(KD // 2)]
                for kp in range(KD // 2):
                    gt = pgd.tile([P, 4, 2, BS], GDT, tag="gd")
                    for gate in range(4):
                        ps = psum_g.tile([P, 2, BS], F32, tag="psg")
                        for half in range(2):
                            m = gate * KD + 2 * kp + half
                            nc.tensor.matmul(ps[:, half, :],
                                             widt[:, m * P:(m + 1) * P],
                                             ytA[t][:, :],
                                             start=True, stop=(t == 0))
                            if t > 0:
                                for k in range(KD):
                                    nc.tensor.matmul(
                                        ps[:, half, :],
                                        whhd[:, k, m * P:(m + 1) * P],
                                        dprev[k // 2][:, k % 2, :],
                                        start=False, stop=(k == KD - 1))
                        fn = AF.Tanh if gate == 2 else AF.Sigmoid
                        nc.scalar.activation(out=gt[:, gate, :, :], in_=ps, func=fn)
                    cs = cd[:, 2 * kp:2 * kp + 2, :]
                    if t == 0:
                        nc.vector.tensor_mul(out=cs, in0=gt[:, 0, :, :],
                                             in1=gt[:, 2, :, :])
                    else:
                        u = pu.tile([P, 2, BS], F32, tag="u")
                        nc.vector.tensor_mul(out=u, in0=gt[:, 0, :, :],
                                             in1=gt[:, 2, :, :])
                        nc.vector.tensor_mul(out=cs, in0=gt[:, 1, :, :], in1=cs)
                        nc.vector.tensor_add(out=cs, in0=cs, in1=u)
                    nc.scalar.activation(out=gt[:, 2, :, :], in_=cs, func=AF.Tanh)
                    nc.vector.tensor_mul(out=dTp[kp], in0=gt[:, 3, :, :],
                                         in1=gt[:, 2, :, :])

        # ---------------- output ----------------
        if upto in ("dec", "dec1", "dec2"):
            nc.vector.tensor_copy(out=osb, in_=cd[:, 0:NJ, 0:1])
        if upto == "full":
            for j in range(NJ):
                psf = psum_f.tile([P, 1], F32, tag="psf")
                for k in range(KD):
                    nc.tensor.matmul(psf, dTp[k // 2][:, k % 2, j * P:(j + 1) * P],
                                     wdt[:, k, :],
                                     start=(k == 0), stop=(k == KD - 1))
                nc.vector.scalar_tensor_tensor(out=osb[:, j, :], in0=psf,
                                               scalar=scal["fcf_b"],
                                               in1=ctxw[:, j:j + 1],
                                               op0=ALU.add, op1=ALU.add)
        nc.sync.dma_start(out=outd.rearrange("(j p) c -> p j c", p=P), in_=osb)

    nc.compile()
    return nc


def _run(inputs, trace=False, upto="full"):
    weights, scal = _pack_weights(inputs)
    nc = _build(scal, upto=upto)
    X = np.ascontiguousarray(_np(inputs["X"]))
    Y = np.ascontiguousarray(_np(inputs["y_prev"]))
    in_maps = []
    for c in range(NCORES):
        m = dict(weights)
        m["x"] = np.ascontiguousarray(X[c * BS:(c + 1) * BS])
        m["y"] = np.ascontiguousarray(Y[c * BS:(c + 1) * BS])
        in_maps.append(m)
    res = run_bass_kernel_spmd(nc, in_maps, core_ids=list(range(NCORES)), trace=trace)
    out = np.concatenate([np.asarray(res.results[i]["out"]) for i in range(NCORES)],
                         axis=0).astype(np.float32)
    return out, res


def kernel(**inputs):
    out, _ = _run(inputs, trace=False)
    return out

